# revision 10
# baseline (speedup 1.0000x reference)
"""Trainium2 Bass kernel for nn_ControllerDecoder (dense_mlp).

Computes, for (in_dim, out_dim, layer) in [(65536,8,1), (8,8,2), (8,65536,3)]:
  w[i,j] = W2 . relu(latent[i] @ W1[:32] + (i/in)W1[32] + (j/out)W1[33] + layer*W1[34] + b1) + b2

Sharding: w1 by i-rows (8192/core), w3 by j-cols (8192/core), w2 replicated.

Per-core device pipeline:
  - TensorE: z tiles [128 h, 1024 pairs] via K-augmented fp16 matmuls
    (position/layer/bias terms folded into extra contraction rows).
  - ScalarE/VectorE (alternating): relu exit PSUM f32 -> SBUF fp16.
  - TensorE: h-reduction with W2: 32 accumulating matmuls per [32,512] PSUM
    tile, each with W2 placed in a different column of a [128,32] stationary
    (other columns zero), so 32 scalar-row results pack one PSUM bank.
  - ScalarE copy [32,512] PSUM->SBUF, DMA to DRAM outputs.

Constraint honored throughout: each Matmult/Ldweights carries at most ONE
semaphore wait (walrus S3_LW limit), via single packed input DMAs and
careful emission order.
"""

import os
import sys
import time

import numpy as np

try:
    import concourse.bass as bass
except ImportError:
    sys.path.insert(0, "/opt/trn_rl_repo")
    import concourse.bass as bass

from contextlib import ExitStack

import concourse.mybir as mybir
import concourse.tile as tile
from concourse import bacc
from concourse.bass_utils import run_bass_kernel_spmd

NCORES = 8
IN_SIZE = 65536
OUT_SIZE = 65536
H = 128
ISH = IN_SIZE // NCORES  # 8192 i-rows per core (w1)
JSH = OUT_SIZE // NCORES  # 8192 j-cols per core (w3)

F16 = mybir.dt.float16
F32 = mybir.dt.float32

# packin fp16 [128, PACKW] column layout
OFF_EMB = 0            # [0:34,     0:8192]   embT (latent.T | pi | ones)
OFF_W1AUG = 8192       # [0:34,  8192:9216]   8x [34,128] w1 stationaries
OFF_W3MOV = 9216       # [0:2,   9216:17408]  (pj ; ones)
OFF_E2 = 17408         # [0:9,  17408:17472]  w2 moving
OFF_W2VAR = 17472      # [0:128,17472:18496]  32x [128,32] W2-in-col-q
OFF_UREP = 18496       # [0:1,  18496:19520]  u=W1[33] replicated 8x
PACKW = 19520

_CACHE = {}
LAST_RESULTS = None


def _build_bass():
    nc = bacc.Bacc(None, target_bir_lowering=False)

    packin_d = nc.dram_tensor("packin", [H, PACKW], F16, kind="ExternalInput")
    headin_d = nc.dram_tensor("headin", [36, 145], F32, kind="ExternalInput")

    w1o_d = nc.dram_tensor("w1o", [128, 512], F32, kind="ExternalOutput")
    w3o_d = nc.dram_tensor("w3o", [128, 512], F32, kind="ExternalOutput")
    w2o_d = nc.dram_tensor("w2o", [1, 64], F32, kind="ExternalOutput")

    w3s_d = nc.dram_tensor("w3s", [2, 8 * H], F16)  # internal DRAM bounce

    with tile.TileContext(nc) as tc, ExitStack() as ctx:
        singles = ctx.enter_context(tc.tile_pool(name="singles", bufs=1))
        relup = ctx.enter_context(tc.tile_pool(name="relup", bufs=3))
        stage = ctx.enter_context(tc.tile_pool(name="stage", bufs=2))
        zpool = ctx.enter_context(tc.tile_pool(name="zpool", bufs=2, space="PSUM"))
        redp = ctx.enter_context(tc.tile_pool(name="redp", bufs=2, space="PSUM"))
        headp = ctx.enter_context(tc.tile_pool(name="headp", bufs=1, space="PSUM"))

        packin = singles.tile([H, PACKW], F16)
        nc.sync.dma_start(out=packin, in_=packin_d[:, :])
        vzero = singles.tile([H, 1], F32)
        nc.vector.memset(vzero, 0.0)
        headin = singles.tile([36, 145], F32)
        nc.sync.dma_start(out=headin, in_=headin_d[:, :])

        xheadT = headin[:, 0:17]
        w1augH = headin[:, 17:145]
        embT = packin[0:34, OFF_EMB:OFF_EMB + ISH]
        w1aug = packin[0:34, OFF_W1AUG:OFF_W1AUG + 8 * H]
        w3mov = packin[0:2, OFF_W3MOV:OFF_W3MOV + JSH]
        e2 = packin[0:9, OFF_E2:OFF_E2 + 64]
        w2var = packin[:, OFF_W2VAR:OFF_W2VAR + 1024]

        # HEAD: out [17, 128] = a2 rows (0-7), u (8), a3 rows (9-16)
        head_ps = headp.tile([17, H], F32, tag="head")
        nc.tensor.matmul(head_ps[:, :], xheadT, w1augH, start=True, stop=True)

        # main dense pipeline (emitted for w1 first, then w3)
        def macro(part, m, red_tiles, out_d, w3lhs=None):
            zp = zpool.tile([H, 1024], F32, tag="z", name=f"z{part}_{m}")
            for s in range(2):
                if part == 0:
                    j, k = divmod(m, 8)
                    lhsT = w1aug[:, j * H:(j + 1) * H]
                    col0 = k * 1024 + s * 512
                    rhs = embT[:, col0:col0 + 512]
                else:
                    i, kj = divmod(m, 8)
                    lhsT = w3lhs[:, i * H:(i + 1) * H]
                    col0 = kj * 1024 + s * 512
                    rhs = w3mov[:, col0:col0 + 512]
                nc.tensor.matmul(zp[:, s * 512:(s + 1) * 512], lhsT, rhs,
                                 start=True, stop=True)
            relu16 = relup.tile([H, 1024], F16, tag="r", name=f"r{part}_{m}")
            if m % 2 == 0:
                nc.scalar.activation(out=relu16, in_=zp,
                                     func=mybir.ActivationFunctionType.Relu)
            else:
                nc.vector.tensor_scalar_max(out=relu16, in0=zp, scalar1=vzero[:, 0:1])
            for s in range(2):
                r = 2 * m + s
                rg, q = divmod(r, 32)
                if q == 0:
                    red_tiles[rg] = redp.tile([32, 512], F32, tag="red",
                                              name=f"red{part}_{rg}")
                nc.tensor.matmul(red_tiles[rg][:, :],
                                 w2var[:, 32 * q:32 * q + 32],
                                 relu16[:, 512 * s:512 * (s + 1)],
                                 start=(q == 0), stop=(q == 31),
                                 skip_group_check=True)
                if q == 31:
                    st = stage.tile([32, 512], F32, tag="st",
                                    name=f"st{part}_{rg}")
                    nc.scalar.copy(out=st, in_=red_tiles[rg][:, :])
                    nc.sync.dma_start(out=out_d[32 * rg:32 * rg + 32, :],
                                      in_=st)

        red1, red3 = {}, {}

        # w1 macro 0 first: establishes PE >= packin ring with 1 wait
        macro(0, 0, red1, w1o_d)

        # head16 copy + w3lhs construction (overlaps w1 work)
        head16 = singles.tile([17, H], F16)
        nc.scalar.copy(out=head16, in_=head_ps)
        # bounce via DRAM so w3lhs has exactly one writer DMA
        nc.sync.dma_start(out=w3s_d[0:1, :], in_=packin_d[0:1, OFF_UREP:OFF_UREP + 8 * H])
        nc.sync.dma_start(out=w3s_d[1:2, :].rearrange("p (i h) -> p i h", i=8),
                          in_=head16[9:17, :])
        w3lhs = singles.tile([2, 8 * H], F16)
        nc.sync.dma_start(out=w3lhs, in_=w3s_d[:, :])

        for m in range(1, 64):
            macro(0, m, red1, w1o_d)

        # w2 (tiny): z2T [128 h, 64 pairs]
        z2 = headp.tile([H, 64], F32, tag="head")
        nc.tensor.matmul(z2[:, :], head16[0:9, :], e2, start=True, stop=True)
        relu2 = singles.tile([H, 64], F16)
        nc.scalar.activation(out=relu2, in_=z2,
                             func=mybir.ActivationFunctionType.Relu)
        w2ps = headp.tile([32, 64], F32, tag="head")
        nc.tensor.matmul(w2ps[:, :], w2var[:, 0:32], relu2[:, :],
                         start=True, stop=True)
        w2sb = singles.tile([1, 64], F32)
        nc.scalar.copy(out=w2sb, in_=w2ps[0:1, :])
        nc.sync.dma_start(out=w2o_d[:, :], in_=w2sb)

        for m in range(64):
            macro(1, m, red3, w3o_d, w3lhs=w3lhs)

    nc.compile()
    return nc


def _host_inputs(latent, W1, b1, W2):
    latent = np.asarray(latent, np.float32)
    W1 = np.asarray(W1, np.float64)
    b1 = np.asarray(b1, np.float64)
    W2 = np.asarray(W2, np.float32)

    iota = np.arange(ISH, dtype=np.float64)

    base = np.zeros((H, PACKW), np.float16)
    # w1aug: 8 blocks [34, 128]
    for j in range(8):
        blk = np.empty((34, H), np.float64)
        blk[:32] = W1[:32]
        blk[32] = W1[32]
        blk[33] = (j / 8.0) * W1[33] + W1[34] + b1
        base[0:34, OFF_W1AUG + j * H:OFF_W1AUG + (j + 1) * H] = blk.astype(
            np.float16)
    # e2
    for i in range(8):
        for j in range(8):
            p = i * 8 + j
            base[i, OFF_E2 + p] = 1.0
            base[8, OFF_E2 + p] = np.float16(j / 8.0)
    # w2var: 32 variants [128, 32], col q = W2
    for q in range(32):
        base[:, OFF_W2VAR + q * 32 + q] = W2[:, 0].astype(np.float16)
    # u replicated
    base[0, OFF_UREP:OFF_UREP + 8 * H] = np.tile(W1[33].astype(np.float16), 8)

    headin = np.zeros((36, 145), np.float32)
    xheadT = headin[:, 0:17]
    xheadT[:32, 0:8] = latent[:8].T
    xheadT[32, 0:8] = np.arange(8) / 8.0
    xheadT[33, 0:8] = 1.0
    xheadT[35, 8] = 1.0
    xheadT[:32, 9:17] = latent[:8].T
    xheadT[32, 9:17] = np.arange(8) / 8.0
    xheadT[33, 9:17] = 1.0
    xheadT[34, 9:17] = 1.0
    headin[:, 17:145] = np.concatenate(
        [W1[:32], W1[32:33], (2.0 * W1[34] + b1)[None], W1[34:35], W1[33:34]],
        0).astype(np.float32)

    in_maps = []
    for c in range(NCORES):
        packin = base.copy()
        i0 = c * ISH
        packin[0:32, OFF_EMB:OFF_EMB + ISH] = latent[i0:i0 + ISH].T.astype(
            np.float16)
        packin[32, OFF_EMB:OFF_EMB + ISH] = (
            (i0 + iota) / float(IN_SIZE)).astype(np.float16)
        packin[33, OFF_EMB:OFF_EMB + ISH] = 1.0
        packin[0, OFF_W3MOV:OFF_W3MOV + JSH] = (
            (c * JSH + iota) / float(OUT_SIZE)).astype(np.float16)
        packin[1, OFF_W3MOV:OFF_W3MOV + JSH] = 1.0
        in_maps.append(dict(packin=packin, headin=headin))
    return in_maps


def kernel(latent, W1, b1, W2, b2, input_size=IN_SIZE, output_size=OUT_SIZE,
           **_ignored):
    global LAST_RESULTS
    if "nc" not in _CACHE:
        _CACHE["nc"] = _build_bass()
    nc = _CACHE["nc"]

    in_maps = _host_inputs(latent, W1, b1, W2)
    trace = bool(int(os.environ.get("KERNEL_TRACE", "0")))
    t0 = time.perf_counter()
    res = run_bass_kernel_spmd(nc, in_maps, core_ids=list(range(NCORES)),
                               trace=trace)
    _CACHE["last_run_s"] = time.perf_counter() - t0
    LAST_RESULTS = res

    b2v = float(np.asarray(b2).ravel()[0])
    w1_parts, w3_parts = [], []
    for c in range(NCORES):
        r = res.results[c]
        # w1o row r = (j*8 + k)*2 + s ; covers i = k*1024 + s*512 + [0,512)
        w1c = (r["w1o"].reshape(8, 8, 2, 512)
               .transpose(1, 2, 3, 0).reshape(ISH, 8))
        w1_parts.append(w1c)
        w3c = r["w3o"].reshape(8, 8, 2, 512).reshape(8, JSH)
        w3_parts.append(w3c)
    w1 = np.concatenate(w1_parts, axis=0)  # [65536, 8]
    w3 = np.concatenate(w3_parts, axis=1)  # [8, 65536]
    w2 = res.results[0]["w2o"].reshape(64)

    out = np.concatenate([w1.ravel(), w2, w3.ravel()]).astype(np.float32)
    if b2v != 0.0:
        out = out + b2v
    return out
